# revision 1
# baseline (speedup 1.0000x reference)
"""Causal self-attention Trainium2 kernel (Bass/Tile), 8 NeuronCores.

Problem: B=2, S=2048, D=1024, H=16 heads (hd=64), fp32.
    qkv = x @ qkv_w + qkv_b ; per-head causal attention ; y = out @ out_w + out_b

Sharding (hybrid data x tensor parallel):
    8 cores = 2 batch groups x 4 head groups. Core c handles batch c//4 and
    the 4 heads [4*(c%4) .. 4*(c%4)+3]. Each core computes its partial
    out-projection y_c [S, D]; host sums the 4 partials per batch + out_b.

Per-core layout strategy (everything contraction-friendly, zero on-chip
transposes):
    - host supplies xT = x[b].T [D, S] so D is the DMA partition dim
    - qkv^T is computed directly: qkT [hd_n on partitions, S free]
    - scores are computed transposed: sT[k, q] = kT.T @ qT, softmax uses no
      max-subtraction (scores are O(6) so exp is safe in fp32), the softmax
      denominator comes out of the PV matmul via a ones-column appended to V,
      and the normalization divides after PV.
    - out^T accumulates in [hd_local=256 partitions, S] layout, which is
      exactly the lhsT the out-projection needs.
Matmuls run as float32r (full-rate fp32 path on TRN2 PE for free dim >= 256).
"""

import os
import sys

for _p in ("/opt/trn_rl_repo", "/root/.axon_site/_ro/trn_rl_repo"):
    if os.path.isdir(_p) and _p not in sys.path:
        sys.path.insert(0, _p)

import numpy as np
from contextlib import ExitStack

import concourse.bass as bass
import concourse.tile as tile
from concourse import bacc, mybir
from concourse.bass_utils import run_bass_kernel_spmd

B, S, D = 2, 2048, 1024
H, HD = 16, 64
NCORES = 8
LOCAL_H = 4           # heads per core
P = 128
KO = D // P           # 8 contraction sub-tiles for the projections
NQ = S // 512         # 4 q-tiles of 512
NKT = S // P          # 16 k-blocks of 128
F32 = mybir.dt.float32
F32R = mybir.dt.float32r
AF = mybir.ActivationFunctionType
SCALE = 1.0 / np.sqrt(HD)


def _emit(tc, nc, xT, wqk, wv, wo, bqkv, b65, onesd, y, has_qkv_bias):
    with ExitStack() as ctx:
        consts = ctx.enter_context(tc.tile_pool(name="consts", bufs=1))
        persis = ctx.enter_context(tc.tile_pool(name="persist", bufs=1))
        psum = ctx.enter_context(tc.tile_pool(name="ps", bufs=2, space="PSUM"))
        psum_o = ctx.enter_context(tc.tile_pool(name="pso", bufs=2, space="PSUM"))
        xstack = ctx.enter_context(ExitStack())
        xpool = xstack.enter_context(tc.tile_pool(name="xp", bufs=KO))

        # ---- constant loads (wqk/x interleaved per-ko so qkT starts early) ----
        b65_sb = consts.tile([1, 260], F32R)
        nc.scalar.dma_start(b65_sb[:], b65[None, :])
        ones_col = consts.tile([1, P], F32R)
        nc.scalar.dma_start(ones_col[:], onesd[None, :])
        # lower-triangle keep-mask for diagonal 128x128 score blocks
        mask128 = consts.tile([P, P], F32R)
        nc.scalar.dma_start(mask128[:], onesd[None, :].to_broadcast((P, P)))
        nc.gpsimd.affine_select(
            out=mask128[:], in_=mask128[:], pattern=[[1, P]],
            compare_op=mybir.AluOpType.is_ge, fill=0.0, base=0,
            channel_multiplier=-1,
        )
        if has_qkv_bias:
            bqk_sb = consts.tile([P, 4], F32)
            nc.scalar.dma_start(bqk_sb[:], bqkv[0:512].rearrange("(m p) -> p m", p=P))

        x_sb, wqk_t, wv_t = [], [], []
        for ko in range(KO):
            w = consts.tile([P, 512], F32R, name=f"wqk{ko}")
            nc.sync.dma_start(w[:], wqk[ko * P:(ko + 1) * P, :])
            wqk_t.append(w)
            t = xpool.tile([P, S], F32R, tag="x")
            nc.sync.dma_start(t[:], xT[ko * P:(ko + 1) * P, :])
            x_sb.append(t)
        for ko in range(KO):
            w = consts.tile([P, 260], F32R, name=f"wv{ko}")
            nc.sync.dma_start(w[:], wv[ko * P:(ko + 1) * P, :])
            wv_t.append(w)
        wo_sb = consts.tile([P, 2, D], F32R)
        nc.sync.dma_start(wo_sb[:], wo.rearrange("(ks p) n -> p ks n", p=P))

        # persistent activations
        qkT = persis.tile([P, 4, S], F32R)       # m-tiles 0,1: qT(h0..h3); 2,3: kT
        v_all = persis.tile([P, NKT, LOCAL_H, 65], F32R)  # [k-part, kt, lh, hd|ones]
        outT = persis.tile([P, 2, S], F32R)      # attention out^T (out-proj lhsT)

        # ---- qk^T projection: qkT[m] = (wqk[:, m-slice]).T @ xT ----
        for m in range(4):
            for n in range(NQ):
                gidx = m * NQ + n
                pool_ = psum if gidx % 2 == 0 else psum_o
                ps = pool_.tile([P, 512], F32, tag="mm512" if gidx % 2 == 0 else "o",
                                name=f"qk{gidx}")
                for ko in range(KO):
                    nc.tensor.matmul(
                        ps[:],
                        (wqk_t[ko][:, m * P:(m + 1) * P]),
                        (x_sb[ko][:, n * 512:(n + 1) * 512]),
                        start=(ko == 0), stop=(ko == KO - 1),
                    )
                dst = qkT[:, m, n * 512:(n + 1) * 512]
                if has_qkv_bias:
                    nc.scalar.activation(dst, ps[:], AF.Identity, bias=bqk_sb[:, m:m + 1])
                else:
                    nc.vector.tensor_copy(dst, ps[:])

        # ---- v projection (natural layout, ones/bias col via K=1 matmul) ----
        for mt in range(NKT):
            pool_ = psum if mt % 2 == 0 else psum_o
            ps = pool_.tile([P, 512], F32, tag="mm512" if mt % 2 == 0 else "o",
                            name=f"vp{mt}")
            pv = ps[:, 0:260]
            for ko in range(KO):
                nc.tensor.matmul(
                    pv,
                    (x_sb[ko][:, mt * P:(mt + 1) * P]),
                    (wv_t[ko][:]),
                    start=(ko == 0), stop=False,
                )
            nc.tensor.matmul(pv, (ones_col[:1, :]), (b65_sb[:1, :]),
                             start=False, stop=True)
            nc.vector.tensor_copy(
                v_all[:, mt, :, :],
                pv.rearrange("p (h d) -> p h d", h=LOCAL_H),
            )

        # x tiles are dead now; release their SBUF for the attention pools
        xstack.close()
        work = ctx.enter_context(tc.tile_pool(name="work", bufs=4))
        small = ctx.enter_context(tc.tile_pool(name="small", bufs=3))

        # ---- attention (jq outer so out-proj can stream per q-tile) ----
        for jq in range(NQ):
            for hp in range(2):        # local heads (2hp, 2hp+1)
                po = [psum_o.tile([65, 512], F32, tag="o", name=f"po{i_}")
                      for i_ in range(2)]
                last_kt = 4 * jq + 3
                for kt in range(last_kt + 1):
                    # diagonal blocks: columns q < 128*rel are fully masked;
                    # compute only [f0, 512) and mask just the 128-wide triangle
                    rel = kt - 4 * jq
                    f0 = 128 * rel if rel > 0 else 0
                    ps = psum.tile([P, 2, 512], F32, tag="s")
                    for i in range(2):
                        poff = 64 * i
                        nc.tensor.matmul(
                            ps[:, i, f0:512],
                            (qkT[poff:poff + 64, 2 + hp, kt * P:(kt + 1) * P]),
                            (qkT[poff:poff + 64, hp,
                                 jq * 512 + f0:(jq + 1) * 512]),
                            start=True, stop=True,
                        )
                    et = work.tile([P, 2, 512], F32R, tag="e")
                    nc.scalar.activation(et[:, :, f0:512], ps[:, :, f0:512],
                                         AF.Exp, scale=float(SCALE))
                    if rel >= 0:   # mask the 128-wide triangle at [f0, f0+128)
                        nc.vector.tensor_tensor(
                            et[:, 0, f0:f0 + 128], et[:, 0, f0:f0 + 128],
                            mask128[:], mybir.AluOpType.mult)
                        nc.vector.tensor_tensor(
                            et[:, 1, f0:f0 + 128], et[:, 1, f0:f0 + 128],
                            mask128[:], mybir.AluOpType.mult)
                    for i in range(2):
                        lh = 2 * hp + i
                        nc.tensor.matmul(
                            po[i][:, f0:512],
                            (v_all[:, kt, lh, :]),
                            (et[:, i, f0:512]),
                            start=(kt == 0), stop=(kt == last_kt),
                        )
                # stage po out of PSUM immediately (frees the bank for the
                # next head-pair), then normalize off-PSUM.
                # 1/l split across engines: i=0 DVE reciprocal, i=1 ACT
                # exp(-ln(l)) (Ln/Exp share the loaded table set).
                for i in range(2):
                    st = work.tile([65, 512], F32, tag="st")
                    nc.vector.tensor_copy(st[:], po[i][:])
                    rr = small.tile([1, 512], F32R, tag="rr")
                    if i == 0:
                        rf = small.tile([1, 512], F32, tag="rf")
                        nc.vector.reciprocal(rf[:], st[64:65, :])
                        nc.vector.tensor_copy(rr[:], rf[:])
                    else:
                        lr = small.tile([1, 512], F32, tag="lr")
                        nc.scalar.activation(lr[:], st[64:65, :], AF.Ln)
                        nc.scalar.activation(rr[:], lr[:], AF.Exp, scale=-1.0)
                    rb_ps = psum_o.tile([64, 512], F32, tag="o", name="rbps")
                    nc.tensor.matmul(rb_ps[:], ones_col[:1, 0:64], rr[:1, :],
                                     start=True, stop=True)
                    nc.vector.tensor_tensor(
                        outT[64 * i:64 * i + 64, hp, jq * 512:(jq + 1) * 512],
                        st[0:64, :], rb_ps[:], mybir.AluOpType.mult,
                    )
            # ---- out-projection for this q-tile's 4 seq sub-tiles ----
            for mt in range(4 * jq, 4 * jq + 4):
                for n2 in range(2):
                    ps = psum.tile([P, 512], F32, tag="mm512")
                    for ks in range(2):
                        nc.tensor.matmul(
                            ps[:],
                            (outT[:, ks, mt * P:(mt + 1) * P]),
                            (wo_sb[:, ks, n2 * 512:(n2 + 1) * 512]),
                            start=(ks == 0), stop=(ks == 1),
                        )
                    yt = work.tile([P, 512], F32, tag="y")
                    nc.vector.tensor_copy(yt[:], ps[:])
                    nc.gpsimd.dma_start(
                        y[mt * P:(mt + 1) * P, n2 * 512:(n2 + 1) * 512], yt[:])


def build_nc(has_qkv_bias):
    nc = bacc.Bacc("TRN2", target_bir_lowering=False, debug=False,
                   num_devices=NCORES)
    xT = nc.dram_tensor("xT", [D, S], F32R, kind="ExternalInput")
    wqk = nc.dram_tensor("wqk", [D, 512], F32R, kind="ExternalInput")
    wv = nc.dram_tensor("wv", [D, 260], F32R, kind="ExternalInput")
    wo = nc.dram_tensor("wo", [2 * P, D], F32R, kind="ExternalInput")
    bqkv = nc.dram_tensor("bqkv", [768], F32, kind="ExternalInput")
    b65 = nc.dram_tensor("b65", [260], F32R, kind="ExternalInput")
    onesd = nc.dram_tensor("onesd", [P], F32R, kind="ExternalInput")
    y = nc.dram_tensor("y", [S, D], F32, kind="ExternalOutput")
    with tile.TileContext(nc) as tc:
        _emit(tc, nc, xT.ap(), wqk.ap(), wv.ap(), wo.ap(), bqkv.ap(), b65.ap(),
              onesd.ap(), y.ap(), has_qkv_bias)
    nc.compile()
    return nc


_NC_CACHE = {}


def _get_nc(has_qkv_bias):
    key = bool(has_qkv_bias)
    if key not in _NC_CACHE:
        _NC_CACHE[key] = build_nc(key)
    return _NC_CACHE[key]


def _round_fp32r(a):
    """Round fp32 to the fp32r grid (11-bit mantissa; low 12 bits zero, RNE)."""
    u = np.ascontiguousarray(a, dtype=np.float32).view(np.uint32)
    u = (u + 0x7FF + ((u >> 12) & 1)) & np.uint32(0xFFFFF000)
    return u.view(np.float32)


def make_in_maps(x, qkv_w, qkv_b, out_w):
    """Per-core host-side sharding. Core c: batch c//4, heads 4*(c%4)..+3."""
    in_maps = []
    xTs = [_round_fp32r(np.ascontiguousarray(x[b].T)) for b in range(B)]
    for c in range(NCORES):
        b = c // (NCORES // B)
        g = c % (NCORES // B)
        h0 = LOCAL_H * g
        cols = slice(h0 * HD, (h0 + LOCAL_H) * HD)
        wq = qkv_w[:, cols]
        wk = qkv_w[:, D:][:, cols]
        wv_ = qkv_w[:, 2 * D:][:, cols]
        bq = qkv_b[cols]
        bk = qkv_b[D:][cols]
        bv = qkv_b[2 * D:][cols]
        wv_pad = np.zeros((D, LOCAL_H, 65), np.float32)
        wv_pad[:, :, :64] = wv_.reshape(D, LOCAL_H, HD)
        b65_arr = np.zeros((LOCAL_H, 65), np.float32)
        b65_arr[:, :64] = bv.reshape(LOCAL_H, HD)
        b65_arr[:, 64] = 1.0
        in_maps.append({
            "xT": xTs[b],
            "wqk": _round_fp32r(np.concatenate([wq, wk], axis=1)),
            "wv": _round_fp32r(wv_pad.reshape(D, LOCAL_H * 65)),
            "wo": _round_fp32r(out_w[cols, :]),
            "bqkv": np.ascontiguousarray(np.concatenate([bq, bk, bv])),
            "b65": _round_fp32r(b65_arr.reshape(-1)),
            "onesd": np.ones(P, np.float32),
        })
    return in_maps


def _ensure_ntff_hook():
    """Provide antenv.axon_hooks (missing in this image) so trace=True works."""
    try:
        from antenv.axon_hooks import get_axon_ntff_profile_hook  # noqa: F401
        return
    except ImportError:
        pass
    import types
    import antenv
    mod = types.ModuleType("antenv.axon_hooks")
    holder = {"hook": None}
    mod.set_axon_ntff_profile_hook = lambda h: holder.__setitem__("hook", h)
    mod.get_axon_ntff_profile_hook = lambda: holder["hook"]
    sys.modules["antenv.axon_hooks"] = mod
    antenv.axon_hooks = mod
    try:
        from trn_agent_boot.trn_boot import _ntff_profile_via_ctypes
        so = "/opt/axon/libaxon_pjrt.so"
        if os.path.exists(so):
            mod.set_axon_ntff_profile_hook(_ntff_profile_via_ctypes(so))
    except Exception:
        pass


def kernel(x, qkv_w, qkv_b, out_w, out_b, _trace=False):
    if _trace:
        _ensure_ntff_hook()
    x = np.asarray(x, dtype=np.float32)
    qkv_w = np.asarray(qkv_w, dtype=np.float32)
    qkv_b = np.asarray(qkv_b, dtype=np.float32)
    out_w = np.asarray(out_w, dtype=np.float32)
    out_b = np.asarray(out_b, dtype=np.float32)

    has_qkv_bias = bool(np.any(qkv_b))
    nc = _get_nc(has_qkv_bias)
    in_maps = make_in_maps(x, qkv_w, qkv_b, out_w)
    res = run_bass_kernel_spmd(nc, in_maps, core_ids=list(range(NCORES)),
                               trace=_trace)
    y = np.zeros((B, S, D), dtype=np.float32)
    for c in range(NCORES):
        y[c // (NCORES // B)] += res.results[c]["y"]
    y += out_b
    if _trace:
        kernel.last_results = res
    return y



# revision 23
# speedup vs baseline: 1.2664x; 1.2664x over previous
"""Causal self-attention Trainium2 kernel (Bass/Tile), 8 NeuronCores.

Problem: B=2, S=2048, D=1024, H=16 heads (hd=64), fp32 in/out.
    qkv = x @ qkv_w + qkv_b ; per-head causal attention ; y = out @ out_w + out_b

Sharding (hybrid data x tensor parallel):
    8 cores = 2 batch groups x 4 head groups. Core c handles batch c//4 and
    the 4 heads [4*(c%4) .. 4*(c%4)+3]. Each core computes its partial
    out-projection y_c [S, D] in fp16; host sums the 4 partials per batch
    (fp32) and adds out_b.

v2 changes vs the fp32r baseline (258us):
    - all matmul operands in fp16 (1 cyc/col on the PE + fast weight load;
      fp32r ran in fp32-HIGH mode at ~2 cyc/col with ~300ns serialized
      weight reloads). PSUM accumulation stays fp32.
    - x is DMA'd in S-slices so the qk-projection starts after ~1/4 of x
      has landed instead of all of it.
    - causal mask applied with gpsimd.affine_select directly on the exp'd
      scores (was: 64 DVE tensor_tensor multiplies with a mask tile).
    - softmax 1/l via vector.reciprocal_approx_fast on a lane-parallel
      [1,2,512] row (was: serial [1,512] DVE reciprocal ~3.3us + Ln/Exp
      table reloads ~1.3us each on the scalar engine).
    - ones column of V written once with memset (was: 16 rank-1 matmuls).
    - scores matmul for kt+1 emitted before PV of kt so the PE never
      waits on the exp chain (in-order engine queues).
"""

import os
import sys

for _p in ("/opt/trn_rl_repo", "/root/.axon_site/_ro/trn_rl_repo"):
    if os.path.isdir(_p) and _p not in sys.path:
        sys.path.insert(0, _p)

import numpy as np
import ml_dtypes
from contextlib import ExitStack

import concourse.bass as bass
import concourse.tile as tile
from concourse import bacc, mybir
from concourse.bass_utils import run_bass_kernel_spmd

B, S, D = 2, 2048, 1024
H, HD = 16, 64
NCORES = 8
LOCAL_H = 4           # heads per core
P = 128
KO = D // P           # 8 contraction sub-tiles for the projections
NQ = S // 512         # 4 q-tiles of 512
NKT = S // P          # 16 k-blocks of 128
F32 = mybir.dt.float32
F16 = mybir.dt.float16
BF16 = mybir.dt.bfloat16
AF = mybir.ActivationFunctionType
ALU = mybir.AluOpType
SCALE = 1.0 / np.sqrt(HD)


def _emit(tc, nc, xt, wqk, wv, wo, bqk, bvb, onesb, onesd, ident, y,
          has_qkv_bias, dbg=None):
    with ExitStack() as ctx:
        consts = ctx.enter_context(tc.tile_pool(name="consts", bufs=1))
        persis = ctx.enter_context(tc.tile_pool(name="persist", bufs=1))

        xstack = ctx.enter_context(ExitStack())
        xpool = xstack.enter_context(tc.tile_pool(name="xp", bufs=32))
        projstack = ctx.enter_context(ExitStack())
        psQK = projstack.enter_context(
            tc.tile_pool(name="psQK", bufs=3, space="PSUM"))
        psV = projstack.enter_context(
            tc.tile_pool(name="psV", bufs=2, space="PSUM"))

        # ---- constants ----
        ones_sb = consts.tile([1, 64], BF16)
        nc.scalar.dma_start(ones_sb[:], onesb[:])
        ident_sb = consts.tile([P, P], BF16)
        nc.scalar.dma_start(ident_sb[:], ident[:])
        if has_qkv_bias:
            bqk_sb = consts.tile([P, 4], F32)
            nc.scalar.dma_start(bqk_sb[:], bqk[:])
            bvb_sb = consts.tile([P, LOCAL_H, HD], F32)
            nc.scalar.dma_start(bvb_sb[:], bvb[:])

        # ---- weight + x loads (interleaved so the first matmul starts
        # after just two chunks) ----
        wqk_sb = consts.tile([P, KO, 512], F16)
        x_sb = [[None] * KO for _ in range(NQ)]
        for ko in range(KO):
            nc.scalar.dma_start(wqk_sb[:, ko, :], wqk[ko])
            t = xpool.tile([P, 512], F16, tag="x", name=f"x0_{ko}")
            nc.sync.dma_start(t[:], xt[0, ko])
            x_sb[0][ko] = t
        wv_sb = consts.tile([P, KO, 256], F16)
        for ko in range(KO):
            nc.scalar.dma_start(wv_sb[:, ko, :], wv[ko])
        for n in range(1, NQ):
            for ko in range(KO):
                t = xpool.tile([P, 512], F16, tag="x", name=f"x{n}_{ko}")
                nc.sync.dma_start(t[:], xt[n, ko])
                x_sb[n][ko] = t
        wo_sb = consts.tile([P, 2, D], F16)
        nc.scalar.dma_start(wo_sb[:], wo[:])

        # persistent activations
        qkT = persis.tile([P, 4, S], F16)        # m 0,1: qT(h0..h3); 2,3: kT
        v_all = persis.tile([P, NKT, LOCAL_H, 65], F16)  # [k-part, kt, lh, hd|1]
        outT = persis.tile([P, 2, S], F16)       # attention out^T (out-proj lhsT)

        nc.scalar.dma_start(v_all[:, :, :, 64:65], onesd[:])

        # ---- qk^T projection, n-slice outer so it starts on x slice 0 ----
        # qkT[m] = (wqk[:, m*128:(m+1)*128]).T @ xT ; m = (q h01, q h23,
        # k h01, k h23)
        copy_flip = 0
        for n in range(NQ):
            ps_a = psQK.tile([P, 2, 512], F32, tag="qk", name=f"qka{n}")
            ps_b = psQK.tile([P, 2, 512], F32, tag="qk", name=f"qkb{n}")
            for ko in range(KO):
                for mh in range(2):
                    nc.tensor.matmul(
                        ps_a[:, mh, :],
                        wqk_sb[:, ko, mh * P:(mh + 1) * P],
                        x_sb[n][ko][:],
                        start=(ko == 0), stop=(ko == KO - 1),
                    )
                    nc.tensor.matmul(
                        ps_b[:, mh, :],
                        wqk_sb[:, ko, 256 + mh * P:256 + (mh + 1) * P],
                        x_sb[n][ko][:],
                        start=(ko == 0), stop=(ko == KO - 1),
                    )
            for mh in range(2):
                for half, m in ((0, mh), (1, 2 + mh)):
                    src = (ps_a if half == 0 else ps_b)[:, mh, :]
                    dst = qkT[:, m, n * 512:(n + 1) * 512]
                    if has_qkv_bias:
                        nc.scalar.activation(dst, src, AF.Identity,
                                             bias=bqk_sb[:, m:m + 1])
                    elif copy_flip % 2 == 0:
                        nc.vector.tensor_copy(dst, src)
                    else:
                        nc.scalar.copy(dst, src)
                    copy_flip += 1

        # ---- v projection (natural [seq, hd] layout) ----
        for mt in range(NKT):
            psv = psV.tile([P, 256], F32, tag="v", name=f"vp{mt}")
            for ko in range(KO):
                nc.tensor.matmul(
                    psv[:],
                    x_sb[mt // 4][ko][:, (mt % 4) * P:(mt % 4 + 1) * P],
                    wv_sb[:, ko, :],
                    start=(ko == 0), stop=(ko == KO - 1),
                )
            src = psv.rearrange("p (h d) -> p h d", h=LOCAL_H)
            dst = v_all[:, mt, :, 0:64]
            if has_qkv_bias:
                nc.vector.tensor_tensor(dst, src, bvb_sb[:], ALU.add)
            elif mt % 2 == 0:
                nc.vector.tensor_copy(dst, src)
            else:
                nc.scalar.copy(dst, src)

        # x tiles and proj psum are dead; release for the attention pools
        projstack.close()
        xstack.close()
        work = ctx.enter_context(tc.tile_pool(name="work", bufs=3))
        small = ctx.enter_context(tc.tile_pool(name="small", bufs=2))
        psS = ctx.enter_context(tc.tile_pool(name="psS", bufs=2, space="PSUM"))
        psO = ctx.enter_context(tc.tile_pool(name="psO", bufs=4, space="PSUM"))

        # ---- attention ----
        # scores transposed: sT[k, q] = kT.T @ qT per head; exp with no max
        # subtraction (logits are O(6)); denominator via the ones column of
        # v_all; po = [65, 512] = (out^T | l) per head.
        for jq in range(NQ):
            pos = {}
            for hp in range(2):        # local head pair (2hp, 2hp+1)
                po = [psO.tile([65, 512], F32, tag="o", name=f"po{jq}{hp}{i_}")
                      for i_ in range(2)]
                pos[hp] = po
                last_kt = 4 * jq + 3

                def emit_scores(kt):
                    rel = kt - 4 * jq
                    f0 = 128 * rel if rel > 0 else 0
                    ps = psS.tile([P, 2, 512], F32, tag="s",
                                  name=f"s{jq}{hp}{kt}")
                    for i in range(2):
                        poff = 64 * i
                        nc.tensor.matmul(
                            ps[:, i, f0:512],
                            qkT[poff:poff + 64, 2 + hp, kt * P:(kt + 1) * P],
                            qkT[poff:poff + 64, hp,
                                jq * 512 + f0:(jq + 1) * 512],
                            start=True, stop=True,
                        )
                    return ps, f0

                prev = emit_scores(0)
                for kt in range(last_kt + 1):
                    ps, f0 = prev
                    rel = kt - 4 * jq
                    et = work.tile([P, 2, 512], F16, tag="e",
                                   name=f"e{jq}{hp}{kt}")
                    nc.scalar.activation(et[:, :, f0:512], ps[:, :, f0:512],
                                         AF.Exp, scale=float(SCALE))
                    if rel >= 0:   # mask the 128-wide diagonal triangle
                        nc.gpsimd.affine_select(
                            out=et[:, :, f0:f0 + 128],
                            in_=et[:, :, f0:f0 + 128],
                            pattern=[[0, 2], [1, P]],
                            compare_op=ALU.is_ge, fill=0.0, base=0,
                            channel_multiplier=-1,
                        )
                    if kt < last_kt:
                        prev = emit_scores(kt + 1)
                    for i in range(2):
                        nc.tensor.matmul(
                            po[i][:, f0:512],
                            v_all[:, kt, 2 * hp + i, :],
                            et[:, i, f0:512],
                            start=(kt == 0), stop=(kt == last_kt),
                        )

            # E1: drain all four po accumulators to SBUF (frees their PSUM
            # banks before the next q-tile's PV loop needs them).
            sts = {}
            for hp in range(2):
                po = pos[hp]
                st = work.tile([65, 2, 512], BF16, tag="st",
                               name=f"st{jq}{hp}")
                nc.vector.tensor_copy(st[:, 0, :], po[0][:])
                nc.scalar.copy(st[:, 1, :], po[1][:])
                sts[hp] = st
            if dbg is not None and jq == 0:
                nc.sync.dma_start(dbg["st00"][:], sts[0][:])
            # E2: softmax 1/l. The l rows sit in the free dim, where a DVE
            # reciprocal is ~6.5 cyc/element; transpose them to partitions
            # with tiny PE transposes, reciprocal lane-parallel ([128,8] =
            # 8 elements/lane), transpose back, and broadcast with a rank-1
            # matmul.
            for hp in range(2):
                st = sts[hp]
                # lT columns are padded to 2 elements so each single-column
                # bf16 PSUM write stays 4-byte aligned.
                lT = psO.tile([P, 8, 2], BF16, tag="o", name=f"lT{jq}{hp}")
                for i in range(2):
                    for qc in range(4):
                        col = i * 4 + qc
                        nc.tensor.transpose(
                            lT[:, col, 0:1],
                            st[64:65, i, qc * 128:(qc + 1) * 128],
                            ident_sb[64:65, 64:65])
                lT32 = small.tile([P, 8], F32, tag="lt32")
                nc.vector.tensor_copy(lT32[:], lT[:, :, 0])
                rrT = small.tile([P, 8], F32, tag="rrt")
                nc.vector.reciprocal(rrT[:], lT32[:])
                rrT16 = small.tile([P, 8], BF16, tag="rrt16")
                nc.vector.tensor_copy(rrT16[:], rrT[:])
                for i in range(2):
                    rr_ps = psO.tile([1, 4, P], BF16, tag="o",
                                     name=f"rrp{jq}{hp}{i}")
                    for qc in range(4):
                        col = i * 4 + qc
                        nc.tensor.transpose(rr_ps[0:1, qc, :],
                                            rrT16[:, col:col + 1],
                                            ident_sb[:, :])
                    rr_sb = small.tile([1, 512], BF16, tag="rrsb",
                                       name=f"rrs{hp}{i}")
                    nc.vector.tensor_copy(
                        rr_sb[:], rr_ps.rearrange("p a b -> p (a b)"))
                    rb = psS.tile([64, 512], F32, tag="s",
                                  name=f"rb{jq}{hp}{i}")
                    nc.tensor.matmul(rb[:], ones_sb[:1, :], rr_sb[:1, :],
                                     start=True, stop=True)
                    nc.vector.tensor_tensor(
                        outT[64 * i:64 * i + 64, hp, jq * 512:(jq + 1) * 512],
                        st[0:64, i, :], rb[:], ALU.mult,
                    )

            # ---- out-projection for this q-tile's 4 seq sub-tiles ----
            for mt in range(4 * jq, 4 * jq + 4):
                pso = psS.tile([P, 2, 512], F32, tag="s", name=f"op{mt}")
                for n2 in range(2):
                    for ks in range(2):
                        nc.tensor.matmul(
                            pso[:, n2, :],
                            outT[:, ks, mt * P:(mt + 1) * P],
                            wo_sb[:, ks, n2 * 512:(n2 + 1) * 512],
                            start=(ks == 0), stop=(ks == 1),
                        )
                yt = work.tile([P, 2, 512], F16, tag="y", name=f"y{mt}")
                nc.vector.tensor_copy(yt[:], pso[:])
                nc.sync.dma_start(
                    y[mt * P:(mt + 1) * P, :],
                    yt.rearrange("p a b -> p (a b)"),
                )
        if dbg is not None:
            nc.sync.dma_start(dbg["qkT"][:], qkT[:])
            nc.sync.dma_start(dbg["v_all"][:], v_all[:])
            nc.sync.dma_start(dbg["outT"][:], outT[:])


def build_nc(has_qkv_bias, debug_dumps=False):
    nc = bacc.Bacc("TRN2", target_bir_lowering=False, debug=False,
                   num_devices=NCORES)
    xt = nc.dram_tensor("xt", [NQ, KO, P, 512], F16, kind="ExternalInput")
    wqk = nc.dram_tensor("wqk", [KO, P, 512], F16, kind="ExternalInput")
    wv = nc.dram_tensor("wv", [KO, P, 256], F16, kind="ExternalInput")
    wo = nc.dram_tensor("wo", [P, 2, D], F16, kind="ExternalInput")
    bqk = nc.dram_tensor("bqk", [P, 4], F32, kind="ExternalInput")
    bvb = nc.dram_tensor("bvb", [P, LOCAL_H, HD], F32, kind="ExternalInput")
    onesb = nc.dram_tensor("onesb", [1, 64], BF16, kind="ExternalInput")
    onesd = nc.dram_tensor("onesd", [P, NKT * LOCAL_H], F16,
                           kind="ExternalInput")
    ident = nc.dram_tensor("ident", [P, P], BF16, kind="ExternalInput")
    y = nc.dram_tensor("y", [S, D], F16, kind="ExternalOutput")
    dbg = None
    if debug_dumps:
        dbg = {
            "qkT": nc.dram_tensor("d_qkT", [P, 4, S], F16,
                                  kind="ExternalOutput").ap(),
            "v_all": nc.dram_tensor("d_vall", [P, NKT, LOCAL_H, 65], F16,
                                    kind="ExternalOutput").ap(),
            "outT": nc.dram_tensor("d_outT", [P, 2, S], F16,
                                   kind="ExternalOutput").ap(),
            "st00": nc.dram_tensor("d_st00", [65, 2, 512], F32,
                                   kind="ExternalOutput").ap(),
        }
    with tile.TileContext(nc) as tc:
        _emit(tc, nc, xt.ap(), wqk.ap(), wv.ap(), wo.ap(), bqk.ap(), bvb.ap(),
              onesb.ap(), onesd.ap(), ident.ap(), y.ap(), has_qkv_bias,
              dbg=dbg)
    nc.compile()
    return nc


_NC_CACHE = {}


def _get_nc(has_qkv_bias):
    key = bool(has_qkv_bias)
    if key not in _NC_CACHE:
        _NC_CACHE[key] = build_nc(key)
    return _NC_CACHE[key]


def make_in_maps(x, qkv_w, qkv_b, out_w):
    """Per-core host-side sharding. Core c: batch c//4, heads 4*(c%4)..+3."""
    in_maps = []
    xts = []
    for b in range(B):
        xT = np.ascontiguousarray(x[b].T).astype(np.float16)      # [D, S]
        xts.append(np.ascontiguousarray(
            xT.reshape(KO, P, NQ, 512).transpose(2, 0, 1, 3)))    # [n, ko, P, 512]
    onesb = np.ones((1, 64), dtype=ml_dtypes.bfloat16)
    for c in range(NCORES):
        b = c // (NCORES // B)
        g = c % (NCORES // B)
        cols = slice(g * LOCAL_H * HD, (g + 1) * LOCAL_H * HD)
        wq = qkv_w[:, 0:D][:, cols]
        wk = qkv_w[:, D:2 * D][:, cols]
        wv_ = qkv_w[:, 2 * D:][:, cols]
        bq = qkv_b[0:D][cols]
        bk = qkv_b[D:2 * D][cols]
        bv = qkv_b[2 * D:][cols]
        wqk_h = np.concatenate([wq, wk], axis=1).astype(np.float16)
        bqk_h = np.stack([bq[0:P], bq[P:256], bk[0:P], bk[P:256]],
                         axis=1).astype(np.float32)
        bvb_h = np.ascontiguousarray(
            np.broadcast_to(bv.reshape(1, LOCAL_H, HD),
                            (P, LOCAL_H, HD))).astype(np.float32)
        in_maps.append({
            "xt": xts[b],
            "wqk": np.ascontiguousarray(wqk_h.reshape(KO, P, 512)),
            "wv": np.ascontiguousarray(
                wv_.astype(np.float16).reshape(KO, P, 256)),
            "wo": np.ascontiguousarray(
                out_w[cols, :].reshape(2, P, D).transpose(1, 0, 2)
                .astype(np.float16)),
            "bqk": bqk_h,
            "bvb": bvb_h,
            "onesb": onesb,
            "onesd": np.ones((P, NKT * LOCAL_H), dtype=np.float16),
            "ident": np.eye(P, dtype=ml_dtypes.bfloat16),
        })
    return in_maps


def _ensure_ntff_hook():
    """Provide antenv.axon_hooks (missing in this image) so trace=True works."""
    try:
        from antenv.axon_hooks import get_axon_ntff_profile_hook  # noqa: F401
        return
    except ImportError:
        pass
    import types
    import antenv
    mod = types.ModuleType("antenv.axon_hooks")
    holder = {"hook": None}
    mod.set_axon_ntff_profile_hook = lambda h: holder.__setitem__("hook", h)
    mod.get_axon_ntff_profile_hook = lambda: holder["hook"]
    sys.modules["antenv.axon_hooks"] = mod
    antenv.axon_hooks = mod
    try:
        from trn_agent_boot.trn_boot import _ntff_profile_via_ctypes
        so = "/opt/axon/libaxon_pjrt.so"
        if os.path.exists(so):
            mod.set_axon_ntff_profile_hook(_ntff_profile_via_ctypes(so))
    except Exception:
        pass


def kernel(x, qkv_w, qkv_b, out_w, out_b, _trace=False):
    if _trace:
        _ensure_ntff_hook()
    x = np.asarray(x, dtype=np.float32)
    qkv_w = np.asarray(qkv_w, dtype=np.float32)
    qkv_b = np.asarray(qkv_b, dtype=np.float32)
    out_w = np.asarray(out_w, dtype=np.float32)
    out_b = np.asarray(out_b, dtype=np.float32)

    has_qkv_bias = bool(np.any(qkv_b))
    nc = _get_nc(has_qkv_bias)
    in_maps = make_in_maps(x, qkv_w, qkv_b, out_w)
    res = run_bass_kernel_spmd(nc, in_maps, core_ids=list(range(NCORES)),
                               trace=_trace)
    y = np.zeros((B, S, D), dtype=np.float32)
    for c in range(NCORES):
        y[c // (NCORES // B)] += res.results[c]["y"].astype(np.float32)
    y += out_b
    if _trace:
        kernel.last_results = res
    return y


# revision 32
# speedup vs baseline: 1.3179x; 1.0407x over previous
"""Causal self-attention Trainium2 kernel (Bass/Tile), 8 NeuronCores.

Problem: B=2, S=2048, D=1024, H=16 heads (hd=64), fp32 in/out.
    qkv = x @ qkv_w + qkv_b ; per-head causal attention ; y = out @ out_w + out_b

Sharding (hybrid data x tensor parallel):
    8 cores = 2 batch groups x 4 head groups. Core c handles batch c//4 and
    the 4 heads [4*(c%4) .. 4*(c%4)+3]. Each core computes its partial
    out-projection y_c [S, D] in fp16; host sums the 4 partials per batch
    (fp32) and adds out_b.

v2 changes vs the fp32r baseline (258us):
    - all matmul operands in fp16 (1 cyc/col on the PE + fast weight load;
      fp32r ran in fp32-HIGH mode at ~2 cyc/col with ~300ns serialized
      weight reloads). PSUM accumulation stays fp32.
    - x is DMA'd in S-slices so the qk-projection starts after ~1/4 of x
      has landed instead of all of it.
    - causal mask applied with gpsimd.affine_select directly on the exp'd
      scores (was: 64 DVE tensor_tensor multiplies with a mask tile).
    - softmax 1/l via vector.reciprocal_approx_fast on a lane-parallel
      [1,2,512] row (was: serial [1,512] DVE reciprocal ~3.3us + Ln/Exp
      table reloads ~1.3us each on the scalar engine).
    - ones column of V written once with memset (was: 16 rank-1 matmuls).
    - scores matmul for kt+1 emitted before PV of kt so the PE never
      waits on the exp chain (in-order engine queues).
"""

import os
import sys

for _p in ("/opt/trn_rl_repo", "/root/.axon_site/_ro/trn_rl_repo"):
    if os.path.isdir(_p) and _p not in sys.path:
        sys.path.insert(0, _p)

import numpy as np
import ml_dtypes
from contextlib import ExitStack

import concourse.bass as bass
import concourse.tile as tile
from concourse import bacc, mybir
from concourse.bass_utils import run_bass_kernel_spmd

B, S, D = 2, 2048, 1024
H, HD = 16, 64
NCORES = 8
LOCAL_H = 4           # heads per core
P = 128
KO = D // P           # 8 contraction sub-tiles for the projections
NQ = S // 512         # 4 q-tiles of 512
NKT = S // P          # 16 k-blocks of 128
F32 = mybir.dt.float32
F16 = mybir.dt.float16
BF16 = mybir.dt.bfloat16
AF = mybir.ActivationFunctionType
ALU = mybir.AluOpType
SCALE = 1.0 / np.sqrt(HD)


def _emit(tc, nc, xt, wqk, wv, wo, bqk, bvb, onesb, onesd, ident, y,
          has_qkv_bias, dbg=None):
    with ExitStack() as ctx:
        consts = ctx.enter_context(tc.tile_pool(name="consts", bufs=1))
        persis = ctx.enter_context(tc.tile_pool(name="persist", bufs=1))

        xstack = ctx.enter_context(ExitStack())
        xpool = xstack.enter_context(tc.tile_pool(name="xp", bufs=32))
        projstack = ctx.enter_context(ExitStack())
        psQK = projstack.enter_context(
            tc.tile_pool(name="psQK", bufs=3, space="PSUM"))
        psV = projstack.enter_context(
            tc.tile_pool(name="psV", bufs=2, space="PSUM"))

        # ---- constants ----
        ones_sb = consts.tile([1, 64], BF16)
        nc.scalar.dma_start(ones_sb[:], onesb[:])
        ident_sb = consts.tile([P, P], BF16)
        nc.scalar.dma_start(ident_sb[:], ident[:])
        if has_qkv_bias:
            bqk_sb = consts.tile([P, 4], F32)
            nc.scalar.dma_start(bqk_sb[:], bqk[:])
            bvb_sb = consts.tile([P, LOCAL_H, HD], F32)
            nc.scalar.dma_start(bvb_sb[:], bvb[:])

        # ---- weight + x loads (interleaved so the first matmul starts
        # after just two chunks) ----
        wqk_sb = consts.tile([P, KO, 512], BF16)
        x_sb = [[None] * KO for _ in range(NQ)]
        for ko in range(KO):
            nc.scalar.dma_start(wqk_sb[:, ko, :], wqk[ko])
            t = xpool.tile([P, 512], BF16, tag="x", name=f"x0_{ko}")
            nc.sync.dma_start(t[:], xt[0, ko])
            x_sb[0][ko] = t
        wv_sb = consts.tile([P, KO, 256], BF16)
        for ko in range(KO):
            nc.scalar.dma_start(wv_sb[:, ko, :], wv[ko])
        for n in range(1, NQ):
            for ko in range(KO):
                t = xpool.tile([P, 512], BF16, tag="x", name=f"x{n}_{ko}")
                nc.sync.dma_start(t[:], xt[n, ko])
                x_sb[n][ko] = t
        wo_sb = consts.tile([P, 2, D], BF16)
        nc.scalar.dma_start(wo_sb[:], wo[:])

        # persistent activations
        qkT = persis.tile([P, 4, S], F16)        # m 0,1: qT(h0..h3); 2,3: kT
        v_all = persis.tile([P, NKT, LOCAL_H, 65], F16)  # [k-part, kt, lh, hd|1]
        outT = persis.tile([P, 2, S], BF16)      # attention out^T (out-proj lhsT)

        nc.scalar.dma_start(v_all[:, :, :, 64:65], onesd[:])

        # ---- qk^T projection, n-slice outer so it starts on x slice 0 ----
        # qkT[m] = (wqk[:, m*128:(m+1)*128]).T @ xT ; m = (q h01, q h23,
        # k h01, k h23)
        copy_flip = 0
        for n in range(NQ):
            ps_a = psQK.tile([P, 2, 512], F32, tag="qk", name=f"qka{n}")
            ps_b = psQK.tile([P, 2, 512], F32, tag="qk", name=f"qkb{n}")
            for ko in range(KO):
                for mh in range(2):
                    nc.tensor.matmul(
                        ps_a[:, mh, :],
                        wqk_sb[:, ko, mh * P:(mh + 1) * P],
                        x_sb[n][ko][:],
                        start=(ko == 0), stop=(ko == KO - 1),
                    )
                    nc.tensor.matmul(
                        ps_b[:, mh, :],
                        wqk_sb[:, ko, 256 + mh * P:256 + (mh + 1) * P],
                        x_sb[n][ko][:],
                        start=(ko == 0), stop=(ko == KO - 1),
                    )
            for mh in range(2):
                for half, m in ((0, mh), (1, 2 + mh)):
                    src = (ps_a if half == 0 else ps_b)[:, mh, :]
                    dst = qkT[:, m, n * 512:(n + 1) * 512]
                    if has_qkv_bias:
                        nc.scalar.activation(dst, src, AF.Identity,
                                             bias=bqk_sb[:, m:m + 1])
                    elif copy_flip % 2 == 0:
                        nc.vector.tensor_copy(dst, src)
                    else:
                        nc.scalar.copy(dst, src)
                    copy_flip += 1

        # ---- v projection (natural [seq, hd] layout) ----
        for mt in range(NKT):
            psv = psV.tile([P, 256], F32, tag="v", name=f"vp{mt}")
            for ko in range(KO):
                nc.tensor.matmul(
                    psv[:],
                    x_sb[mt // 4][ko][:, (mt % 4) * P:(mt % 4 + 1) * P],
                    wv_sb[:, ko, :],
                    start=(ko == 0), stop=(ko == KO - 1),
                )
            src = psv.rearrange("p (h d) -> p h d", h=LOCAL_H)
            dst = v_all[:, mt, :, 0:64]
            if has_qkv_bias:
                nc.vector.tensor_tensor(dst, src, bvb_sb[:], ALU.add)
            elif mt % 2 == 0:
                nc.vector.tensor_copy(dst, src)
            else:
                nc.scalar.copy(dst, src)

        # x tiles and proj psum are dead; release for the attention pools
        projstack.close()
        xstack.close()
        work = ctx.enter_context(tc.tile_pool(name="work", bufs=3))
        small = ctx.enter_context(tc.tile_pool(name="small", bufs=2))
        psS = ctx.enter_context(tc.tile_pool(name="psS", bufs=2, space="PSUM"))
        psO = ctx.enter_context(tc.tile_pool(name="psO", bufs=4, space="PSUM"))

        # ---- attention ----
        # scores transposed: sT[k, q] = kT.T @ qT per head; exp with no max
        # subtraction (logits are O(6)); denominator via the ones column of
        # v_all; po = [65, 512] = (out^T | l) per head.
        for jq in range(NQ):
            pos = {}
            for hp in range(2):        # local head pair (2hp, 2hp+1)
                po = [psO.tile([65, 512], F32, tag="o", name=f"po{jq}{hp}{i_}")
                      for i_ in range(2)]
                pos[hp] = po
                last_kt = 4 * jq + 3

                def emit_scores(kt):
                    rel = kt - 4 * jq
                    f0 = 128 * rel if rel > 0 else 0
                    ps = psS.tile([P, 2, 512], F32, tag="s",
                                  name=f"s{jq}{hp}{kt}")
                    for i in range(2):
                        poff = 64 * i
                        nc.tensor.matmul(
                            ps[:, i, f0:512],
                            qkT[poff:poff + 64, 2 + hp, kt * P:(kt + 1) * P],
                            qkT[poff:poff + 64, hp,
                                jq * 512 + f0:(jq + 1) * 512],
                            start=True, stop=True,
                        )
                    return ps, f0

                prev = emit_scores(0)
                for kt in range(last_kt + 1):
                    ps, f0 = prev
                    rel = kt - 4 * jq
                    et = work.tile([P, 2, 512], F16, tag="e",
                                   name=f"e{jq}{hp}{kt}")
                    nc.scalar.activation(et[:, :, f0:512], ps[:, :, f0:512],
                                         AF.Exp, scale=float(SCALE))
                    if rel >= 0:   # mask the 128-wide diagonal triangle
                        nc.gpsimd.affine_select(
                            out=et[:, :, f0:f0 + 128],
                            in_=et[:, :, f0:f0 + 128],
                            pattern=[[0, 2], [1, P]],
                            compare_op=ALU.is_ge, fill=0.0, base=0,
                            channel_multiplier=-1,
                        )
                    if kt < last_kt:
                        prev = emit_scores(kt + 1)
                    for i in range(2):
                        nc.tensor.matmul(
                            po[i][:, f0:512],
                            v_all[:, kt, 2 * hp + i, :],
                            et[:, i, f0:512],
                            start=(kt == 0), stop=(kt == last_kt),
                        )

            # E1: drain all four po accumulators to SBUF (frees their PSUM
            # banks before the next q-tile's PV loop needs them).
            sts = {}
            for hp in range(2):
                po = pos[hp]
                st = work.tile([65, 2, 512], BF16, tag="st",
                               name=f"st{jq}{hp}")
                nc.vector.tensor_copy(st[:, 0, :], po[0][:])
                nc.vector.tensor_copy(st[:, 1, :], po[1][:])
                sts[hp] = st
            if dbg is not None and jq == 0:
                nc.sync.dma_start(dbg["st00"][:], sts[0][:])
            # E2: softmax 1/l. The l rows sit in the free dim, where a DVE
            # reciprocal is ~6.5 cyc/element; transpose them to partitions
            # with tiny PE transposes, reciprocal lane-parallel ([128,8] =
            # 8 elements/lane), transpose back, and broadcast with a rank-1
            # matmul.
            # forward transposes + recip chain for both head pairs first, so
            # each PE step has DVE work already in flight to hide behind.
            rrT16s = {}
            for hp in range(2):
                st = sts[hp]
                # lT columns are padded to 2 elements so each single-column
                # bf16 PSUM write stays 4-byte aligned.
                lT = psO.tile([P, 8, 2], BF16, tag="o", name=f"lT{jq}{hp}")
                for i in range(2):
                    for qc in range(4):
                        col = i * 4 + qc
                        nc.tensor.transpose(
                            lT[:, col, 0:1],
                            st[64:65, i, qc * 128:(qc + 1) * 128],
                            ident_sb[64:65, 64:65])
                lT32 = small.tile([P, 8], F32, tag="lt32")
                nc.vector.tensor_copy(lT32[:], lT[:, :, 0])
                rrT = small.tile([P, 8], F32, tag="rrt")
                nc.vector.reciprocal(rrT[:], lT32[:])
                rrT16 = small.tile([P, 8], BF16, tag="rrt16",
                                   name=f"rrT16{jq}{hp}")
                nc.vector.tensor_copy(rrT16[:], rrT[:])
                rrT16s[hp] = rrT16
            # back transposes / rank-1 broadcasts / normalize, interleaved so
            # every rank-1 sits two PE ops after its rr_sb copy.
            rr_sbs = {}
            for hp in range(2):
                for i in range(2):
                    rr_ps = psO.tile([1, 4, P], BF16, tag="o",
                                     name=f"rrp{jq}{hp}{i}")
                    for qc in range(4):
                        col = i * 4 + qc
                        nc.tensor.transpose(rr_ps[0:1, qc, :],
                                            rrT16s[hp][:, col:col + 1],
                                            ident_sb[:, :])
                    rr_sb = small.tile([1, 512], BF16, tag="rrsb",
                                       name=f"rrs{hp}{i}")
                    nc.vector.tensor_copy(
                        rr_sb[:], rr_ps.rearrange("p a b -> p (a b)"))
                    rr_sbs[(hp, i)] = rr_sb
            for hp in range(2):
                for i in range(2):
                    rb = psS.tile([64, 512], F32, tag="s",
                                  name=f"rb{jq}{hp}{i}")
                    nc.tensor.matmul(rb[:], ones_sb[:1, :],
                                     rr_sbs[(hp, i)][:1, :],
                                     start=True, stop=True)
                    nc.vector.tensor_tensor(
                        outT[64 * i:64 * i + 64, hp, jq * 512:(jq + 1) * 512],
                        sts[hp][0:64, i, :], rb[:], ALU.mult,
                    )

            # ---- out-projection for this q-tile's 4 seq sub-tiles ----
            for mt in range(4 * jq, 4 * jq + 4):
                pso = psS.tile([P, 2, 512], F32, tag="s", name=f"op{mt}")
                for ks in range(2):
                    for n2 in range(2):
                        nc.tensor.matmul(
                            pso[:, n2, :],
                            outT[:, ks, mt * P:(mt + 1) * P],
                            wo_sb[:, ks, n2 * 512:(n2 + 1) * 512],
                            start=(ks == 0), stop=(ks == 1),
                        )
                yt = work.tile([P, 2, 512], F16, tag="y", name=f"y{mt}")
                nc.vector.tensor_copy(yt[:], pso[:])
                nc.sync.dma_start(
                    y[mt * P:(mt + 1) * P, :],
                    yt.rearrange("p a b -> p (a b)"),
                )
        if dbg is not None:
            nc.sync.dma_start(dbg["qkT"][:], qkT[:])
            nc.sync.dma_start(dbg["v_all"][:], v_all[:])
            nc.sync.dma_start(dbg["outT"][:], outT[:])


def build_nc(has_qkv_bias, debug_dumps=False):
    nc = bacc.Bacc("TRN2", target_bir_lowering=False, debug=False,
                   num_devices=NCORES)
    xt = nc.dram_tensor("xt", [NQ, KO, P, 512], BF16, kind="ExternalInput")
    wqk = nc.dram_tensor("wqk", [KO, P, 512], BF16, kind="ExternalInput")
    wv = nc.dram_tensor("wv", [KO, P, 256], BF16, kind="ExternalInput")
    wo = nc.dram_tensor("wo", [P, 2, D], BF16, kind="ExternalInput")
    bqk = nc.dram_tensor("bqk", [P, 4], F32, kind="ExternalInput")
    bvb = nc.dram_tensor("bvb", [P, LOCAL_H, HD], F32, kind="ExternalInput")
    onesb = nc.dram_tensor("onesb", [1, 64], BF16, kind="ExternalInput")
    onesd = nc.dram_tensor("onesd", [P, NKT * LOCAL_H], F16,
                           kind="ExternalInput")
    ident = nc.dram_tensor("ident", [P, P], BF16, kind="ExternalInput")
    y = nc.dram_tensor("y", [S, D], F16, kind="ExternalOutput")
    dbg = None
    if debug_dumps:
        dbg = {
            "qkT": nc.dram_tensor("d_qkT", [P, 4, S], F16,
                                  kind="ExternalOutput").ap(),
            "v_all": nc.dram_tensor("d_vall", [P, NKT, LOCAL_H, 65], F16,
                                    kind="ExternalOutput").ap(),
            "outT": nc.dram_tensor("d_outT", [P, 2, S], F16,
                                   kind="ExternalOutput").ap(),
            "st00": nc.dram_tensor("d_st00", [65, 2, 512], F32,
                                   kind="ExternalOutput").ap(),
        }
    with tile.TileContext(nc) as tc:
        _emit(tc, nc, xt.ap(), wqk.ap(), wv.ap(), wo.ap(), bqk.ap(), bvb.ap(),
              onesb.ap(), onesd.ap(), ident.ap(), y.ap(), has_qkv_bias,
              dbg=dbg)
    nc.compile()
    return nc


_NC_CACHE = {}


def _get_nc(has_qkv_bias):
    key = bool(has_qkv_bias)
    if key not in _NC_CACHE:
        _NC_CACHE[key] = build_nc(key)
    return _NC_CACHE[key]


def make_in_maps(x, qkv_w, qkv_b, out_w):
    """Per-core host-side sharding. Core c: batch c//4, heads 4*(c%4)..+3."""
    in_maps = []
    xts = []
    for b in range(B):
        xT = np.ascontiguousarray(x[b].T).astype(ml_dtypes.bfloat16)  # [D, S]
        xts.append(np.ascontiguousarray(
            xT.reshape(KO, P, NQ, 512).transpose(2, 0, 1, 3)))    # [n, ko, P, 512]
    onesb = np.ones((1, 64), dtype=ml_dtypes.bfloat16)
    for c in range(NCORES):
        b = c // (NCORES // B)
        g = c % (NCORES // B)
        cols = slice(g * LOCAL_H * HD, (g + 1) * LOCAL_H * HD)
        wq = qkv_w[:, 0:D][:, cols]
        wk = qkv_w[:, D:2 * D][:, cols]
        wv_ = qkv_w[:, 2 * D:][:, cols]
        bq = qkv_b[0:D][cols]
        bk = qkv_b[D:2 * D][cols]
        bv = qkv_b[2 * D:][cols]
        wqk_h = np.concatenate([wq, wk], axis=1).astype(ml_dtypes.bfloat16)
        bqk_h = np.stack([bq[0:P], bq[P:256], bk[0:P], bk[P:256]],
                         axis=1).astype(np.float32)
        bvb_h = np.ascontiguousarray(
            np.broadcast_to(bv.reshape(1, LOCAL_H, HD),
                            (P, LOCAL_H, HD))).astype(np.float32)
        in_maps.append({
            "xt": xts[b],
            "wqk": np.ascontiguousarray(wqk_h.reshape(KO, P, 512)),
            "wv": np.ascontiguousarray(
                wv_.astype(ml_dtypes.bfloat16).reshape(KO, P, 256)),
            "wo": np.ascontiguousarray(
                out_w[cols, :].reshape(2, P, D).transpose(1, 0, 2)
                .astype(ml_dtypes.bfloat16)),
            "bqk": bqk_h,
            "bvb": bvb_h,
            "onesb": onesb,
            "onesd": np.ones((P, NKT * LOCAL_H), dtype=np.float16),
            "ident": np.eye(P, dtype=ml_dtypes.bfloat16),
        })
    return in_maps


def _ensure_ntff_hook():
    """Provide antenv.axon_hooks (missing in this image) so trace=True works."""
    try:
        from antenv.axon_hooks import get_axon_ntff_profile_hook  # noqa: F401
        return
    except ImportError:
        pass
    import types
    import antenv
    mod = types.ModuleType("antenv.axon_hooks")
    holder = {"hook": None}
    mod.set_axon_ntff_profile_hook = lambda h: holder.__setitem__("hook", h)
    mod.get_axon_ntff_profile_hook = lambda: holder["hook"]
    sys.modules["antenv.axon_hooks"] = mod
    antenv.axon_hooks = mod
    try:
        from trn_agent_boot.trn_boot import _ntff_profile_via_ctypes
        so = "/opt/axon/libaxon_pjrt.so"
        if os.path.exists(so):
            mod.set_axon_ntff_profile_hook(_ntff_profile_via_ctypes(so))
    except Exception:
        pass


def kernel(x, qkv_w, qkv_b, out_w, out_b, _trace=False):
    if _trace:
        _ensure_ntff_hook()
    x = np.asarray(x, dtype=np.float32)
    qkv_w = np.asarray(qkv_w, dtype=np.float32)
    qkv_b = np.asarray(qkv_b, dtype=np.float32)
    out_w = np.asarray(out_w, dtype=np.float32)
    out_b = np.asarray(out_b, dtype=np.float32)

    has_qkv_bias = bool(np.any(qkv_b))
    nc = _get_nc(has_qkv_bias)
    in_maps = make_in_maps(x, qkv_w, qkv_b, out_w)
    res = run_bass_kernel_spmd(nc, in_maps, core_ids=list(range(NCORES)),
                               trace=_trace)
    y = np.zeros((B, S, D), dtype=np.float32)
    for c in range(NCORES):
        y[c // (NCORES // B)] += res.results[c]["y"].astype(np.float32)
    y += out_b
    if _trace:
        kernel.last_results = res
    return y


# revision 36
# speedup vs baseline: 1.3825x; 1.0490x over previous
"""Causal self-attention Trainium2 kernel (Bass/Tile), 8 NeuronCores.

Problem: B=2, S=2048, D=1024, H=16 heads (hd=64), fp32 in/out.
    qkv = x @ qkv_w + qkv_b ; per-head causal attention ; y = out @ out_w + out_b

Sharding (hybrid data x tensor parallel):
    8 cores = 2 batch groups x 4 head groups. Core c handles batch c//4 and
    the 4 heads [4*(c%4) .. 4*(c%4)+3]. Each core computes its partial
    out-projection y_c [S, D] in fp16; host sums the 4 partials per batch
    (fp32) and adds out_b.

v2 changes vs the fp32r baseline (258us):
    - all matmul operands in fp16 (1 cyc/col on the PE + fast weight load;
      fp32r ran in fp32-HIGH mode at ~2 cyc/col with ~300ns serialized
      weight reloads). PSUM accumulation stays fp32.
    - x is DMA'd in S-slices so the qk-projection starts after ~1/4 of x
      has landed instead of all of it.
    - causal mask applied with gpsimd.affine_select directly on the exp'd
      scores (was: 64 DVE tensor_tensor multiplies with a mask tile).
    - softmax 1/l via vector.reciprocal_approx_fast on a lane-parallel
      [1,2,512] row (was: serial [1,512] DVE reciprocal ~3.3us + Ln/Exp
      table reloads ~1.3us each on the scalar engine).
    - ones column of V written once with memset (was: 16 rank-1 matmuls).
    - scores matmul for kt+1 emitted before PV of kt so the PE never
      waits on the exp chain (in-order engine queues).
"""

import os
import sys

for _p in ("/opt/trn_rl_repo", "/root/.axon_site/_ro/trn_rl_repo"):
    if os.path.isdir(_p) and _p not in sys.path:
        sys.path.insert(0, _p)

import numpy as np
import ml_dtypes
from contextlib import ExitStack

import concourse.bass as bass
import concourse.tile as tile
from concourse import bacc, mybir
from concourse.bass_utils import run_bass_kernel_spmd

B, S, D = 2, 2048, 1024
H, HD = 16, 64
NCORES = 8
LOCAL_H = 4           # heads per core
P = 128
KO = D // P           # 8 contraction sub-tiles for the projections
NQ = S // 512         # 4 q-tiles of 512
NKT = S // P          # 16 k-blocks of 128
F32 = mybir.dt.float32
F16 = mybir.dt.float16
BF16 = mybir.dt.bfloat16
AF = mybir.ActivationFunctionType
ALU = mybir.AluOpType
SCALE = 1.0 / np.sqrt(HD)


def _emit(tc, nc, xt, wqk, wv, wo, bqk, bvb, onesb, onesd, ident, y,
          has_qkv_bias, dbg=None):
    with ExitStack() as ctx:
        consts = ctx.enter_context(tc.tile_pool(name="consts", bufs=1))
        persis = ctx.enter_context(tc.tile_pool(name="persist", bufs=1))

        xstack = ctx.enter_context(ExitStack())
        xpool = xstack.enter_context(tc.tile_pool(name="xp", bufs=32))
        projstack = ctx.enter_context(ExitStack())
        psQK = projstack.enter_context(
            tc.tile_pool(name="psQK", bufs=3, space="PSUM"))
        psV = projstack.enter_context(
            tc.tile_pool(name="psV", bufs=2, space="PSUM"))

        # ---- constants ----
        ones_sb = consts.tile([1, 64], BF16)
        nc.scalar.dma_start(ones_sb[:], onesb[:])
        ident_sb = consts.tile([P, P], BF16)
        nc.scalar.dma_start(ident_sb[:], ident[:])
        if has_qkv_bias:
            bqk_sb = consts.tile([P, 4], F32)
            nc.scalar.dma_start(bqk_sb[:], bqk[:])
            bvb_sb = consts.tile([P, LOCAL_H, HD], F32)
            nc.scalar.dma_start(bvb_sb[:], bvb[:])

        # ---- weight + x loads (interleaved so the first matmul starts
        # after just two chunks) ----
        wqk_sb = consts.tile([P, KO, 512], BF16)
        x_sb = [[None] * KO for _ in range(NQ)]
        for ko in range(KO):
            nc.scalar.dma_start(wqk_sb[:, ko, :], wqk[ko])
            t = xpool.tile([P, 512], BF16, tag="x", name=f"x0_{ko}")
            nc.sync.dma_start(t[:], xt[0, ko])
            x_sb[0][ko] = t
        wv_sb = consts.tile([P, KO, 256], BF16)
        for ko in range(KO):
            nc.scalar.dma_start(wv_sb[:, ko, :], wv[ko])
        for n in range(1, NQ):
            for ko in range(KO):
                t = xpool.tile([P, 512], BF16, tag="x", name=f"x{n}_{ko}")
                nc.sync.dma_start(t[:], xt[n, ko])
                x_sb[n][ko] = t
        wo_sb = consts.tile([P, 2, D], BF16)
        nc.scalar.dma_start(wo_sb[:], wo[:])

        # persistent activations
        qkT = persis.tile([P, 4, S], F16)        # m 0,1: qT(h0..h3); 2,3: kT
        v_all = persis.tile([P, NKT, LOCAL_H, 65], F16)  # [k-part, kt, lh, hd|1]
        outT = persis.tile([P, 2, S], BF16)      # attention out^T (out-proj lhsT)

        nc.scalar.dma_start(v_all[:, :, :, 64:65], onesd[:])

        # ---- qk^T projection, n-slice outer so it starts on x slice 0 ----
        # qkT[m] = (wqk[:, m*128:(m+1)*128]).T @ xT ; m = (q h01, q h23,
        # k h01, k h23)
        copy_flip = 0
        for n in range(NQ):
            ps_a = psQK.tile([P, 2, 512], F32, tag="qk", name=f"qka{n}")
            ps_b = psQK.tile([P, 2, 512], F32, tag="qk", name=f"qkb{n}")
            for ko in range(KO):
                for mh in range(2):
                    nc.tensor.matmul(
                        ps_a[:, mh, :],
                        wqk_sb[:, ko, mh * P:(mh + 1) * P],
                        x_sb[n][ko][:],
                        start=(ko == 0), stop=(ko == KO - 1),
                    )
                    nc.tensor.matmul(
                        ps_b[:, mh, :],
                        wqk_sb[:, ko, 256 + mh * P:256 + (mh + 1) * P],
                        x_sb[n][ko][:],
                        start=(ko == 0), stop=(ko == KO - 1),
                    )
            for mh in range(2):
                for half, m in ((0, mh), (1, 2 + mh)):
                    src = (ps_a if half == 0 else ps_b)[:, mh, :]
                    dst = qkT[:, m, n * 512:(n + 1) * 512]
                    if has_qkv_bias:
                        nc.scalar.activation(dst, src, AF.Identity,
                                             bias=bqk_sb[:, m:m + 1])
                    elif copy_flip % 2 == 0:
                        nc.vector.tensor_copy(dst, src)
                    else:
                        nc.scalar.copy(dst, src)
                    copy_flip += 1

        # ---- v projection (natural [seq, hd] layout) ----
        for mt in range(NKT):
            psv = psV.tile([P, 256], F32, tag="v", name=f"vp{mt}")
            for ko in range(KO):
                nc.tensor.matmul(
                    psv[:],
                    x_sb[mt // 4][ko][:, (mt % 4) * P:(mt % 4 + 1) * P],
                    wv_sb[:, ko, :],
                    start=(ko == 0), stop=(ko == KO - 1),
                )
            src = psv.rearrange("p (h d) -> p h d", h=LOCAL_H)
            dst = v_all[:, mt, :, 0:64]
            if has_qkv_bias:
                nc.vector.tensor_tensor(dst, src, bvb_sb[:], ALU.add)
            elif mt % 2 == 0:
                nc.vector.tensor_copy(dst, src)
            else:
                nc.scalar.copy(dst, src)

        # x tiles and proj psum are dead; release for the attention pools
        projstack.close()
        xstack.close()
        work = ctx.enter_context(tc.tile_pool(name="work", bufs=3))
        small = ctx.enter_context(tc.tile_pool(name="small", bufs=2))
        psS = ctx.enter_context(tc.tile_pool(name="psS", bufs=2, space="PSUM"))
        psO = ctx.enter_context(tc.tile_pool(name="psO", bufs=4, space="PSUM"))

        # ---- attention ----
        # scores transposed: sT[k, q] = kT.T @ qT per head; exp with no max
        # subtraction (logits are O(6)); denominator via the ones column of
        # v_all; po = [65, 512] = (out^T | l) per head.

        def emit_outproj(jq):
            # out-projection for q-tile jq's 4 seq sub-tiles. Emitted one
            # q-tile behind the attention loop so its PE work covers the
            # epilogue's DVE chain.
            for mt in range(4 * jq, 4 * jq + 4):
                pso = psS.tile([P, 2, 512], F32, tag="s", name=f"op{mt}")
                for ks in range(2):
                    for n2 in range(2):
                        nc.tensor.matmul(
                            pso[:, n2, :],
                            outT[:, ks, mt * P:(mt + 1) * P],
                            wo_sb[:, ks, n2 * 512:(n2 + 1) * 512],
                            start=(ks == 0), stop=(ks == 1),
                        )
                yt = work.tile([P, 2, 512], F16, tag="y", name=f"y{mt}")
                nc.vector.tensor_copy(yt[:], pso[:])
                nc.sync.dma_start(
                    y[mt * P:(mt + 1) * P, :],
                    yt.rearrange("p a b -> p (a b)"),
                )

        for jq in range(NQ):
            pos = {}
            for hp in range(2):        # local head pair (2hp, 2hp+1)
                po = [psO.tile([65, 512], F32, tag="o", name=f"po{jq}{hp}{i_}")
                      for i_ in range(2)]
                pos[hp] = po
                last_kt = 4 * jq + 3

                def emit_scores(kt):
                    rel = kt - 4 * jq
                    f0 = 128 * rel if rel > 0 else 0
                    ps = psS.tile([P, 2, 512], F32, tag="s",
                                  name=f"s{jq}{hp}{kt}")
                    for i in range(2):
                        poff = 64 * i
                        nc.tensor.matmul(
                            ps[:, i, f0:512],
                            qkT[poff:poff + 64, 2 + hp, kt * P:(kt + 1) * P],
                            qkT[poff:poff + 64, hp,
                                jq * 512 + f0:(jq + 1) * 512],
                            start=True, stop=True,
                        )
                    return ps, f0

                prev = emit_scores(0)
                for kt in range(last_kt + 1):
                    ps, f0 = prev
                    rel = kt - 4 * jq
                    et = work.tile([P, 2, 512], F16, tag="e",
                                   name=f"e{jq}{hp}{kt}")
                    nc.scalar.activation(et[:, :, f0:512], ps[:, :, f0:512],
                                         AF.Exp, scale=float(SCALE))
                    if rel >= 0:   # mask the 128-wide diagonal triangle
                        nc.gpsimd.affine_select(
                            out=et[:, :, f0:f0 + 128],
                            in_=et[:, :, f0:f0 + 128],
                            pattern=[[0, 2], [1, P]],
                            compare_op=ALU.is_ge, fill=0.0, base=0,
                            channel_multiplier=-1,
                        )
                    if kt < last_kt:
                        prev = emit_scores(kt + 1)
                    for i in range(2):
                        nc.tensor.matmul(
                            po[i][:, f0:512],
                            v_all[:, kt, 2 * hp + i, :],
                            et[:, i, f0:512],
                            start=(kt == 0), stop=(kt == last_kt),
                        )

            # E1: drain all four po accumulators to SBUF (frees their PSUM
            # banks before the next q-tile's PV loop needs them).
            sts = {}
            for hp in range(2):
                po = pos[hp]
                st = work.tile([65, 2, 512], BF16, tag="st",
                               name=f"st{jq}{hp}")
                nc.vector.tensor_copy(st[:, 0, :], po[0][:])
                nc.vector.tensor_copy(st[:, 1, :], po[1][:])
                sts[hp] = st
            if dbg is not None and jq == 0:
                nc.sync.dma_start(dbg["st00"][:], sts[0][:])
            # E2: softmax 1/l. The l rows sit in the free dim, where a DVE
            # reciprocal is ~6.5 cyc/element; transpose them to partitions
            # with tiny PE transposes, reciprocal lane-parallel ([128,8] =
            # 8 elements/lane), transpose back, and broadcast with a rank-1
            # matmul.
            if jq > 0:
                emit_outproj(jq - 1)
            # forward transposes + recip chain for both head pairs first, so
            # each PE step has DVE work already in flight to hide behind.
            rrT16s = {}
            for hp in range(2):
                st = sts[hp]
                # lT columns are padded to 2 elements so each single-column
                # bf16 PSUM write stays 4-byte aligned.
                lT = psO.tile([P, 8, 2], BF16, tag="o", name=f"lT{jq}{hp}")
                for i in range(2):
                    for qc in range(4):
                        col = i * 4 + qc
                        nc.tensor.transpose(
                            lT[:, col, 0:1],
                            st[64:65, i, qc * 128:(qc + 1) * 128],
                            ident_sb[64:65, 64:65])
                lT32 = small.tile([P, 8], F32, tag="lt32")
                nc.vector.tensor_copy(lT32[:], lT[:, :, 0])
                rrT = small.tile([P, 8], F32, tag="rrt")
                nc.vector.reciprocal(rrT[:], lT32[:])
                rrT16 = small.tile([P, 8], BF16, tag="rrt16",
                                   name=f"rrT16{jq}{hp}")
                nc.vector.tensor_copy(rrT16[:], rrT[:])
                rrT16s[hp] = rrT16
            # back transposes, then broadcast 1/l down 64 partitions on the
            # (otherwise idle) gpsimd engine and normalize on the DVE.
            for hp in range(2):
                for i in range(2):
                    rr_ps = psO.tile([1, 4, P], BF16, tag="o",
                                     name=f"rrp{jq}{hp}{i}")
                    for qc in range(4):
                        col = i * 4 + qc
                        nc.tensor.transpose(rr_ps[0:1, qc, :],
                                            rrT16s[hp][:, col:col + 1],
                                            ident_sb[:, :])
                    rr_sb = small.tile([1, 512], BF16, tag="rrsb",
                                       name=f"rrs{hp}{i}")
                    nc.vector.tensor_copy(
                        rr_sb[:], rr_ps.rearrange("p a b -> p (a b)"))
                    rbs = small.tile([64, 512], BF16, tag="rbs",
                                     name=f"rbs{hp}{i}")
                    nc.gpsimd.partition_broadcast(rbs[:], rr_sb[:])
                    nc.vector.tensor_tensor(
                        outT[64 * i:64 * i + 64, hp, jq * 512:(jq + 1) * 512],
                        sts[hp][0:64, i, :], rbs[:], ALU.mult,
                    )

        emit_outproj(NQ - 1)
        if dbg is not None:
            nc.sync.dma_start(dbg["qkT"][:], qkT[:])
            nc.sync.dma_start(dbg["v_all"][:], v_all[:])
            nc.sync.dma_start(dbg["outT"][:], outT[:])


def build_nc(has_qkv_bias, debug_dumps=False):
    nc = bacc.Bacc("TRN2", target_bir_lowering=False, debug=False,
                   num_devices=NCORES)
    xt = nc.dram_tensor("xt", [NQ, KO, P, 512], BF16, kind="ExternalInput")
    wqk = nc.dram_tensor("wqk", [KO, P, 512], BF16, kind="ExternalInput")
    wv = nc.dram_tensor("wv", [KO, P, 256], BF16, kind="ExternalInput")
    wo = nc.dram_tensor("wo", [P, 2, D], BF16, kind="ExternalInput")
    bqk = nc.dram_tensor("bqk", [P, 4], F32, kind="ExternalInput")
    bvb = nc.dram_tensor("bvb", [P, LOCAL_H, HD], F32, kind="ExternalInput")
    onesb = nc.dram_tensor("onesb", [1, 64], BF16, kind="ExternalInput")
    onesd = nc.dram_tensor("onesd", [P, NKT * LOCAL_H], F16,
                           kind="ExternalInput")
    ident = nc.dram_tensor("ident", [P, P], BF16, kind="ExternalInput")
    y = nc.dram_tensor("y", [S, D], F16, kind="ExternalOutput")
    dbg = None
    if debug_dumps:
        dbg = {
            "qkT": nc.dram_tensor("d_qkT", [P, 4, S], F16,
                                  kind="ExternalOutput").ap(),
            "v_all": nc.dram_tensor("d_vall", [P, NKT, LOCAL_H, 65], F16,
                                    kind="ExternalOutput").ap(),
            "outT": nc.dram_tensor("d_outT", [P, 2, S], F16,
                                   kind="ExternalOutput").ap(),
            "st00": nc.dram_tensor("d_st00", [65, 2, 512], F32,
                                   kind="ExternalOutput").ap(),
        }
    with tile.TileContext(nc) as tc:
        _emit(tc, nc, xt.ap(), wqk.ap(), wv.ap(), wo.ap(), bqk.ap(), bvb.ap(),
              onesb.ap(), onesd.ap(), ident.ap(), y.ap(), has_qkv_bias,
              dbg=dbg)
    nc.compile()
    return nc


_NC_CACHE = {}


def _get_nc(has_qkv_bias):
    key = bool(has_qkv_bias)
    if key not in _NC_CACHE:
        _NC_CACHE[key] = build_nc(key)
    return _NC_CACHE[key]


def make_in_maps(x, qkv_w, qkv_b, out_w):
    """Per-core host-side sharding. Core c: batch c//4, heads 4*(c%4)..+3."""
    in_maps = []
    xts = []
    for b in range(B):
        xT = np.ascontiguousarray(x[b].T).astype(ml_dtypes.bfloat16)  # [D, S]
        xts.append(np.ascontiguousarray(
            xT.reshape(KO, P, NQ, 512).transpose(2, 0, 1, 3)))    # [n, ko, P, 512]
    onesb = np.ones((1, 64), dtype=ml_dtypes.bfloat16)
    for c in range(NCORES):
        b = c // (NCORES // B)
        g = c % (NCORES // B)
        cols = slice(g * LOCAL_H * HD, (g + 1) * LOCAL_H * HD)
        wq = qkv_w[:, 0:D][:, cols]
        wk = qkv_w[:, D:2 * D][:, cols]
        wv_ = qkv_w[:, 2 * D:][:, cols]
        bq = qkv_b[0:D][cols]
        bk = qkv_b[D:2 * D][cols]
        bv = qkv_b[2 * D:][cols]
        wqk_h = np.concatenate([wq, wk], axis=1).astype(ml_dtypes.bfloat16)
        bqk_h = np.stack([bq[0:P], bq[P:256], bk[0:P], bk[P:256]],
                         axis=1).astype(np.float32)
        bvb_h = np.ascontiguousarray(
            np.broadcast_to(bv.reshape(1, LOCAL_H, HD),
                            (P, LOCAL_H, HD))).astype(np.float32)
        in_maps.append({
            "xt": xts[b],
            "wqk": np.ascontiguousarray(wqk_h.reshape(KO, P, 512)),
            "wv": np.ascontiguousarray(
                wv_.astype(ml_dtypes.bfloat16).reshape(KO, P, 256)),
            "wo": np.ascontiguousarray(
                out_w[cols, :].reshape(2, P, D).transpose(1, 0, 2)
                .astype(ml_dtypes.bfloat16)),
            "bqk": bqk_h,
            "bvb": bvb_h,
            "onesb": onesb,
            "onesd": np.ones((P, NKT * LOCAL_H), dtype=np.float16),
            "ident": np.eye(P, dtype=ml_dtypes.bfloat16),
        })
    return in_maps


def _ensure_ntff_hook():
    """Provide antenv.axon_hooks (missing in this image) so trace=True works."""
    try:
        from antenv.axon_hooks import get_axon_ntff_profile_hook  # noqa: F401
        return
    except ImportError:
        pass
    import types
    import antenv
    mod = types.ModuleType("antenv.axon_hooks")
    holder = {"hook": None}
    mod.set_axon_ntff_profile_hook = lambda h: holder.__setitem__("hook", h)
    mod.get_axon_ntff_profile_hook = lambda: holder["hook"]
    sys.modules["antenv.axon_hooks"] = mod
    antenv.axon_hooks = mod
    try:
        from trn_agent_boot.trn_boot import _ntff_profile_via_ctypes
        so = "/opt/axon/libaxon_pjrt.so"
        if os.path.exists(so):
            mod.set_axon_ntff_profile_hook(_ntff_profile_via_ctypes(so))
    except Exception:
        pass


def kernel(x, qkv_w, qkv_b, out_w, out_b, _trace=False):
    if _trace:
        _ensure_ntff_hook()
    x = np.asarray(x, dtype=np.float32)
    qkv_w = np.asarray(qkv_w, dtype=np.float32)
    qkv_b = np.asarray(qkv_b, dtype=np.float32)
    out_w = np.asarray(out_w, dtype=np.float32)
    out_b = np.asarray(out_b, dtype=np.float32)

    has_qkv_bias = bool(np.any(qkv_b))
    nc = _get_nc(has_qkv_bias)
    in_maps = make_in_maps(x, qkv_w, qkv_b, out_w)
    res = run_bass_kernel_spmd(nc, in_maps, core_ids=list(range(NCORES)),
                               trace=_trace)
    y = np.zeros((B, S, D), dtype=np.float32)
    for c in range(NCORES):
        y[c // (NCORES // B)] += res.results[c]["y"].astype(np.float32)
    y += out_b
    if _trace:
        kernel.last_results = res
    return y
